# revision 6
# baseline (speedup 1.0000x reference)
"""Bidirectional cross-attention + conv fusion block on 8 Trainium2 NeuronCores.

Sharding: data-parallel over the 8 independent (sample, direction) attention
units — core c handles sample c//2, direction c%2 (0 = s2-query, 1 = dem-query).
After attention + channel-LayerNorm, core pairs AllGather their LN outputs
(= the channel concat), every core computes the full 3x3 conv for its sample,
BatchNorm statistics are AllReduced across one core per sample, and each core
finishes BN + ReLU + 1x1 conv for its sample. Host takes even cores' outputs.

Precision: fp32r (single-pass fp32, ~2^-13 rounding) for projections / logits /
LN / final 1x1 matmuls; bf16 for the exp'd attention matrix P, the AV matmuls,
and the 3x3 conv. Softmax needs no max-subtraction: |logits| <~ 1 by
construction (weights ~N(0, 0.05^2)).
"""
import numpy as np
import ml_dtypes
from contextlib import ExitStack

import concourse.bass as bass
import concourse.tile as tile
from concourse import bacc, mybir
from concourse.bass_utils import run_bass_kernel_spmd

F32 = mybir.dt.float32
F32R = mybir.dt.float32r
BF16 = mybir.dt.bfloat16
Exp = mybir.ActivationFunctionType.Exp
Sqrt = mybir.ActivationFunctionType.Sqrt
MULT = mybir.AluOpType.mult
ADD = mybir.AluOpType.add
AX = mybir.AxisListType.X

B, C, H, W = 4, 64, 64, 64
HW = H * W            # 4096
N_CORES = 8
EPS_LN = 1e-5
EPS_BN = 1e-5
NT = HW // 512        # 8 j-tiles of 512
NI = HW // 128        # 32 i-blocks of 128
BN_COUNT = float(B * HW)

AG_GROUPS = [[0, 1], [2, 3], [4, 5], [6, 7]]
AR_GROUPS = [[0, 2, 4, 6], [1, 3, 5, 7]]

_CACHE = {}


def _build():
    nc = bacc.Bacc("TRN2", target_bir_lowering=False, debug=False,
                   num_devices=N_CORES)

    def din(name, shape, dt):
        return nc.dram_tensor(name, shape, dt, kind="ExternalInput").ap()

    xa_d = din("xa", [C, HW], F32R)          # query-side input (own direction)
    xb_d = din("xb", [C, HW], F32R)          # key/value-side input
    wq_d = din("wq", [C, 2 * C], F32R)       # wq.T duplicated along M
    wk_d = din("wk", [C, 2 * C], F32R)       # wk.T duplicated
    wvT_d = din("wvT", [C, C], F32R)         # wv.T
    bq_d = din("bq", [2 * C, 1], F32)        # bq duplicated along partitions
    bk_d = din("bk", [2 * C, 1], F32)
    bv_d = din("bv", [2 * C, C], F32)        # bv broadcast across partitions
    lnm_d = din("lnm", [C, C], F32R)         # all-1/64 (channel-mean matmul)
    lng_d = din("lng", [C, 1], F32)          # LN weight (own direction)
    lnb_d = din("lnb", [C, 1], F32)          # LN bias
    fw1_d = din("fw1t", [2 * C, 9 * C], BF16)  # conv w: [ic, tap*oc]
    fb1_d = din("fb1", [C, 1], F32)
    bng_d = din("bng", [C, 1], F32)
    bnb_d = din("bnb", [C, 1], F32)
    fw2_d = din("fw2T", [C, C], F32R)        # fw2.T
    fb2_d = din("fb2", [C, 1], F32)

    out_d = nc.dram_tensor("out", [C, HW], F32, kind="ExternalOutput").ap()

    ag_in = nc.dram_tensor("ag_in", [C, HW], BF16).ap()
    ag_out = nc.dram_tensor("ag_out", [2 * C, HW], BF16).ap()
    ar_in = nc.dram_tensor("ar_in", [C, 2], F32).ap()
    ar_out = nc.dram_tensor("ar_out", [C, 2], F32).ap()

    with tile.TileContext(nc) as tc:
        with ExitStack() as ctx:
            const = ctx.enter_context(tc.tile_pool(name="const", bufs=1))
            big = ctx.enter_context(tc.tile_pool(name="big", bufs=1))
            small = ctx.enter_context(tc.tile_pool(name="small", bufs=2))
            lps = ctx.enter_context(tc.tile_pool(name="lps", bufs=4, space="PSUM"))
            acc = ctx.enter_context(tc.tile_pool(name="acc", bufs=1, space="PSUM"))
            actx = ExitStack()
            abig = actx.enter_context(tc.tile_pool(name="abig", bufs=1))
            ppool = actx.enter_context(tc.tile_pool(name="ppool", bufs=2))

            # ---- load inputs ----
            xa = const.tile([C, HW], F32R, tag="xa")
            xb = abig.tile([C, HW], F32R, tag="xb")
            nc.sync.dma_start(xa[:], xa_d[:])
            nc.sync.dma_start(xb[:], xb_d[:])
            wq = const.tile([C, 2 * C], F32R, tag="wq")
            wk = const.tile([C, 2 * C], F32R, tag="wk")
            wvT = const.tile([C, C], F32R, tag="wvT")
            nc.sync.dma_start(wq[:], wq_d[:])
            nc.sync.dma_start(wk[:], wk_d[:])
            nc.sync.dma_start(wvT[:], wvT_d[:])
            bq = const.tile([2 * C, 1], F32, tag="bq")
            bk = const.tile([2 * C, 1], F32, tag="bk")
            bv = const.tile([2 * C, C], F32, tag="bv")
            nc.sync.dma_start(bq[:], bq_d[:])
            nc.sync.dma_start(bk[:], bk_d[:])
            nc.sync.dma_start(bv[:], bv_d[:])
            lnm = const.tile([C, C], F32R, tag="lnm")
            lng = const.tile([C, 1], F32, tag="lng")
            lnb = const.tile([C, 1], F32, tag="lnb")
            nc.sync.dma_start(lnm[:], lnm_d[:])
            nc.sync.dma_start(lng[:], lng_d[:])
            nc.sync.dma_start(lnb[:], lnb_d[:])
            fw1 = const.tile([2 * C, 9, C], BF16, tag="fw1")
            nc.sync.dma_start(fw1[:], fw1_d[:].rearrange("p (t o) -> p t o", t=9))
            fb1 = const.tile([C, 1], F32, tag="fb1")
            bng = const.tile([C, 1], F32, tag="bng")
            bnb = const.tile([C, 1], F32, tag="bnb")
            fw2 = const.tile([C, C], F32R, tag="fw2")
            fb2 = const.tile([C, 1], F32, tag="fb2")
            nc.sync.dma_start(fb1[:], fb1_d[:])
            nc.sync.dma_start(bng[:], bng_d[:])
            nc.sync.dma_start(bnb[:], bnb_d[:])
            nc.sync.dma_start(fw2[:], fw2_d[:])
            nc.sync.dma_start(fb2[:], fb2_d[:])

            eps = const.tile([C, 1], F32, tag="eps")
            nc.vector.memset(eps[:], EPS_LN)

            # ---- projections: Q2/K2 duplicated on both partition halves ----
            Q2 = abig.tile([2 * C, HW], F32R, tag="Q2")
            K2 = abig.tile([2 * C, HW], F32R, tag="K2")
            for jt in range(NT):
                sl = slice(jt * 512, (jt + 1) * 512)
                pq = lps.tile([2 * C, 512], F32, tag="lps")
                nc.tensor.matmul(pq[:], wq[:], xa[:, sl])
                nc.vector.tensor_scalar_add(Q2[:, sl], pq[:], bq[:])
                pk = lps.tile([2 * C, 512], F32, tag="lps")
                nc.tensor.matmul(pk[:], wk[:], xb[:, sl])
                nc.vector.tensor_scalar_add(K2[:, sl], pk[:], bk[:])

            # vT[i, c] with +bv broadcast along free dim
            VT = abig.tile([128, NI, C], F32, tag="VT")
            for ib in range(NI):
                pv = lps.tile([128, C], F32, tag="lps")
                nc.tensor.matmul(pv[:], xb[:, ib * 128:(ib + 1) * 128], wvT[:])
                nc.vector.tensor_add(VT[:, ib, :], pv[:], bv[:])

            # ---- main attention loop over i-block pairs ----
            accb = [acc.tile([128, 512], F32, tag=f"acc{jj}", name=f"acc{jj}")
                    for jj in range(4)]
            for ibp in range(NI // 2):
                iA, iB = 2 * ibp, 2 * ibp + 1
                PA = ppool.tile([128, HW], BF16, tag="PA")
                PB = ppool.tile([128, HW], BF16, tag="PB")
                sp = small.tile([128, 16], F32, tag="sp")
                for jt in range(NT):
                    sl = slice(jt * 512, (jt + 1) * 512)
                    psA = lps.tile([128, 512], F32, tag="lps")
                    psB = lps.tile([128, 512], F32, tag="lps")
                    nc.tensor.matmul(psA[:], K2[0:C, iA * 128:(iA + 1) * 128],
                                     Q2[0:C, sl], tile_position=(0, 0))
                    nc.tensor.matmul(psB[:], K2[C:2 * C, iB * 128:(iB + 1) * 128],
                                     Q2[C:2 * C, sl], tile_position=(64, 0))
                    nc.scalar.activation(PA[:, sl], psA[:], Exp, scale=0.125,
                                         accum_out=sp[:, jt:jt + 1])
                    nc.scalar.activation(PB[:, sl], psB[:], Exp, scale=0.125,
                                         accum_out=sp[:, 8 + jt:8 + jt + 1])
                sAB = small.tile([128, 2], F32, tag="sAB")
                nc.vector.tensor_reduce(sAB[:, 0:1], sp[:, 0:8], AX, ADD)
                nc.vector.tensor_reduce(sAB[:, 1:2], sp[:, 8:16], AX, ADD)
                rAB = small.tile([128, 2], F32, tag="rAB")
                nc.vector.reciprocal(rAB[:], sAB[:])
                vsA = small.tile([128, C], BF16, tag="vsA")
                vsB = small.tile([128, C], BF16, tag="vsB")
                nc.vector.tensor_scalar_mul(vsA[:], VT[:, iA, :], rAB[:, 0:1])
                nc.vector.tensor_scalar_mul(vsB[:], VT[:, iB, :], rAB[:, 1:2])
                first, last = ibp == 0, ibp == NI // 2 - 1
                for jj in range(4):
                    se = slice((2 * jj) * 512, (2 * jj + 1) * 512)
                    so = slice((2 * jj + 1) * 512, (2 * jj + 2) * 512)
                    nc.tensor.matmul(accb[jj][0:C, :], vsA[:], PA[:, se],
                                     tile_position=(0, 0), start=first, stop=False)
                    nc.tensor.matmul(accb[jj][C:2 * C, :], vsA[:], PA[:, so],
                                     tile_position=(0, 64), start=first, stop=False)
                    nc.tensor.matmul(accb[jj][0:C, :], vsB[:], PB[:, se],
                                     tile_position=(0, 0), start=False, stop=last)
                    nc.tensor.matmul(accb[jj][C:2 * C, :], vsB[:], PB[:, so],
                                     tile_position=(0, 64), start=False, stop=last)

            # ---- residual + channel LayerNorm ----
            att = big.tile([C, HW], F32R, tag="att")
            for jj in range(4):
                for half in range(2):
                    jt = 2 * jj + half
                    sl = slice(jt * 512, (jt + 1) * 512)
                    nc.vector.tensor_add(att[:, sl],
                                         accb[jj][half * C:(half + 1) * C, :],
                                         xa[:, sl])

            actx.close()
            tmp = ctx.enter_context(tc.tile_pool(name="tmp", bufs=2))
            oln = big.tile([C, HW], BF16, tag="oln")
            for jt in range(NT):
                sl = slice(jt * 512, (jt + 1) * 512)
                sq = tmp.tile([C, 512], F32R, tag="sq")
                nc.vector.tensor_mul(sq[:], att[:, sl], att[:, sl])
                pmu = lps.tile([C, 512], F32, tag="lps")
                pe2 = lps.tile([C, 512], F32, tag="lps")
                nc.tensor.matmul(pmu[:], lnm[:], att[:, sl])   # E[x] replicated
                nc.tensor.matmul(pe2[:], lnm[:], sq[:])        # E[x^2] replicated
                mucp = tmp.tile([C, 512], F32, tag="mucp")
                nc.vector.tensor_copy(mucp[:], pmu[:])
                musq = tmp.tile([C, 512], F32, tag="musq")
                nc.vector.tensor_mul(musq[:], mucp[:], mucp[:])
                varr = tmp.tile([C, 512], F32, tag="varr")
                nc.vector.tensor_sub(varr[:], pe2[:], musq[:])
                sd = tmp.tile([C, 512], F32, tag="sd")
                nc.scalar.activation(sd[:], varr[:], Sqrt, bias=eps[:])
                rstd = tmp.tile([C, 512], F32, tag="rstd")
                nc.vector.reciprocal(rstd[:], sd[:])
                xmu = tmp.tile([C, 512], F32, tag="xmu")
                nc.vector.tensor_sub(xmu[:], att[:, sl], mucp[:])
                xh = tmp.tile([C, 512], F32, tag="xh")
                nc.vector.tensor_mul(xh[:], xmu[:], rstd[:])
                nc.vector.tensor_scalar(oln[:, sl], xh[:], lng[:], lnb[:],
                                        MULT, ADD)

            # ---- pair exchange: channel concat via AllGather ----
            nc.sync.dma_start(ag_in[:], oln[:])
            nc.gpsimd.collective_compute("AllGather", mybir.AluOpType.bypass,
                                         replica_groups=AG_GROUPS,
                                         ins=[ag_in[:]], outs=[ag_out[:]])

            # ---- 3x3 conv on full sample (zero-padded 66x66 buffer) ----
            pad = big.tile([2 * C, H + 2, W + 2], BF16, tag="pad")
            nc.vector.memset(pad[:], 0.0)
            nc.sync.dma_start(pad[:, 1:H + 1, 1:W + 1],
                              ag_out[:].rearrange("p (h w) -> p h w", h=H))

            y = big.tile([C, HW], F32, tag="y")
            for g in range(8):   # 8 row-groups of 8 rows
                pc = lps.tile([C, 8, W], F32, tag="lps")
                t = 0
                for ki in range(3):
                    for kj in range(3):
                        nc.tensor.matmul(
                            pc[:], fw1[:, 3 * ki + kj, :],
                            pad[:, 8 * g + ki:8 * g + ki + 8, kj:kj + W],
                            start=(t == 0), stop=(t == 8))
                        t += 1
                nc.vector.tensor_scalar_add(
                    y[:, g * 512:(g + 1) * 512],
                    pc[:].rearrange("p r w -> p (r w)"), fb1[:])

            # ---- BatchNorm stats (cross-sample AllReduce) ----
            bnp = small.tile([C, 2], F32, tag="bnp")
            nc.vector.tensor_reduce(bnp[:, 0:1], y[:], AX, ADD)
            ysq = big.tile([C, HW], F32, tag="ysq")
            nc.vector.tensor_mul(ysq[:], y[:], y[:])
            nc.vector.tensor_reduce(bnp[:, 1:2], ysq[:], AX, ADD)
            nc.sync.dma_start(ar_in[:], bnp[:])
            nc.gpsimd.collective_compute("AllReduce", mybir.AluOpType.add,
                                         replica_groups=AR_GROUPS,
                                         ins=[ar_in[:]], outs=[ar_out[:]])
            bns = small.tile([C, 2], F32, tag="bns")
            nc.sync.dma_start(bns[:], ar_out[:])

            m2 = small.tile([C, 2], F32, tag="m2")
            nc.vector.tensor_scalar_mul(m2[:], bns[:], 1.0 / BN_COUNT)
            musq2 = small.tile([C, 1], F32, tag="musq2")
            nc.vector.tensor_mul(musq2[:], m2[:, 0:1], m2[:, 0:1])
            varb = small.tile([C, 1], F32, tag="varb")
            nc.vector.tensor_sub(varb[:], m2[:, 1:2], musq2[:])
            sdb = small.tile([C, 1], F32, tag="sdb")
            nc.scalar.activation(sdb[:], varb[:], Sqrt, bias=eps[:])
            rstdb = small.tile([C, 1], F32, tag="rstdb")
            nc.vector.reciprocal(rstdb[:], sdb[:])
            scl = small.tile([C, 1], F32, tag="scl")
            nc.vector.tensor_mul(scl[:], bng[:], rstdb[:])
            msc = small.tile([C, 1], F32, tag="msc")
            nc.vector.tensor_mul(msc[:], m2[:, 0:1], scl[:])
            shf = small.tile([C, 1], F32, tag="shf")
            nc.vector.tensor_sub(shf[:], bnb[:], msc[:])

            # ---- BN apply + ReLU + final 1x1 conv ----
            yr = big.tile([C, HW], F32R, tag="yr")
            yb = big.tile([C, HW], F32, tag="yb")
            nc.vector.tensor_scalar(yb[:], y[:], scl[:], shf[:], MULT, ADD)
            nc.vector.tensor_scalar_max(yr[:], yb[:], 0.0)
            for jt in range(NT):
                sl = slice(jt * 512, (jt + 1) * 512)
                po = lps.tile([C, 512], F32, tag="lps")
                nc.tensor.matmul(po[:], fw2[:], yr[:, sl])
                ot = tmp.tile([C, 512], F32, tag="ot")
                nc.vector.tensor_scalar_add(ot[:], po[:], fb2[:])
                nc.sync.dma_start(out_d[:, sl], ot[:])

    nc.compile()
    return nc


def _get_nc():
    if "nc" not in _CACHE:
        _CACHE["nc"] = _build()
    return _CACHE["nc"]


def kernel(x_s2, x_dem, wq1, bq1, wk1, bk1, wv1, bv1,
           wq2, bq2, wk2, bk2, wv2, bv2,
           ln_s2_w, ln_s2_b, ln_dem_w, ln_dem_b,
           fw1, fb1, bn_g, bn_b, fw2, fb2):
    nc = _get_nc()
    f32 = np.float32
    x_s2 = np.asarray(x_s2, f32).reshape(B, C, HW)
    x_dem = np.asarray(x_dem, f32).reshape(B, C, HW)

    def dup_w(w):       # [64,64] -> wT duplicated along M: [64,128]
        wT = np.ascontiguousarray(np.asarray(w, f32).T)
        return np.concatenate([wT, wT], axis=1)

    def dup_b(b):
        bb = np.asarray(b, f32).reshape(C)
        return np.concatenate([bb, bb]).reshape(2 * C, 1)

    fw1t = np.ascontiguousarray(
        np.transpose(np.asarray(fw1, f32), (1, 2, 3, 0)).reshape(2 * C, 9 * C)
    ).astype(ml_dtypes.bfloat16)
    lnm = np.full((C, C), 1.0 / C, f32)
    common = {
        "lnm": lnm,
        "fw1t": fw1t,
        "fb1": np.asarray(fb1, f32).reshape(C, 1),
        "bng": np.asarray(bn_g, f32).reshape(C, 1),
        "bnb": np.asarray(bn_b, f32).reshape(C, 1),
        "fw2T": np.ascontiguousarray(np.asarray(fw2, f32).T),
        "fb2": np.asarray(fb2, f32).reshape(C, 1),
    }
    dir_params = [
        dict(wq=dup_w(wq1), wk=dup_w(wk1), wvT=np.ascontiguousarray(np.asarray(wv1, f32).T),
             bq=dup_b(bq1), bk=dup_b(bk1),
             bv=np.tile(np.asarray(bv1, f32).reshape(1, C), (2 * C, 1)),
             lng=np.asarray(ln_s2_w, f32).reshape(C, 1),
             lnb=np.asarray(ln_s2_b, f32).reshape(C, 1)),
        dict(wq=dup_w(wq2), wk=dup_w(wk2), wvT=np.ascontiguousarray(np.asarray(wv2, f32).T),
             bq=dup_b(bq2), bk=dup_b(bk2),
             bv=np.tile(np.asarray(bv2, f32).reshape(1, C), (2 * C, 1)),
             lng=np.asarray(ln_dem_w, f32).reshape(C, 1),
             lnb=np.asarray(ln_dem_b, f32).reshape(C, 1)),
    ]
    in_maps = []
    for c in range(N_CORES):
        b, d = c // 2, c % 2
        xa = x_s2[b] if d == 0 else x_dem[b]
        xbv = x_dem[b] if d == 0 else x_s2[b]
        m = {"xa": np.ascontiguousarray(xa), "xb": np.ascontiguousarray(xbv)}
        m.update(dir_params[d])
        m.update(common)
        in_maps.append(m)

    res = run_bass_kernel_spmd(nc, in_maps, list(range(N_CORES)))
    out = np.empty((B, C, H, W), np.float32)
    for b in range(B):
        out[b] = res.results[2 * b]["out"].reshape(C, H, W)
    return out
